# revision 27
# baseline (speedup 1.0000x reference)
"""
MLA attention (DeepSeek-style) on 8 TRN2 NeuronCores.

Sharding:
  phase 1 (LoRA-A projection + RMSNorm): sharded over sequence (256 rows/core),
    result transposed to feature-major and AllGathered (bf16 latents).
    The kv+rope latent columns are computed first and gathered in an early
    collective that overlaps the rest of phase 1; the q latents follow.
  phase 2 (q/kv up-proj, attention, o_proj): sharded over heads (4 heads/core),
    w_o input-dim sharded; partial outputs summed on the host (the all-reduce).

All heavy matmuls run in bf16 with fp32 PSUM accumulation.
Everything feature-major ("X^T" layout [feature, seq]) in phase 2 so no big
transposes are needed.

Perf notes vs the first working version:
  - w_qkv_a is repacked on the host into contiguous [128, cw] tiles and
    streamed on BOTH HWDGE queues (sync + scalar); the strided layout capped
    the single queue at ~150 GB/s and starved the PE through phase 1.
  - collective input DMAs + triggers go through the gpsimd queue so they are
    not stuck behind the weight stream; q-latent readback is split across the
    two HWDGE queues which are idle by then.
  - w_q_b is column-reordered on the host (per-head nope blocks, then
    rope-pairs of two heads) so every up-proj matmul uses the full 128-wide
    PE array.
  - softmax denominators: exp tiles are pre-summed in groups of 4 on the DVE,
    one ones-matmul per group instead of per tile (4x fewer PE columns).
  - causal diagonal tiles only stream their live column range.
  - the reciprocal-broadcast matmul runs in fp16 (it was silently fp32 =
    4 cycles/col); the renormalization multiply reads both PSUM operands
    directly via scalar_tensor_tensor.
  - o_proj partials are written out in bf16 (host sums in fp64).
"""

import os
import sys
from contextlib import ExitStack

import numpy as np

for _p in ("/opt/trn_rl_repo", "/root/.axon_site/_ro/trn_rl_repo"):
    if os.path.isdir(_p) and _p not in sys.path:
        sys.path.insert(0, _p)

import ml_dtypes  # noqa: E402

import concourse.bacc as bacc  # noqa: E402
import concourse.bass as bass  # noqa: E402
import concourse.mybir as mybir  # noqa: E402
import concourse.tile as tile  # noqa: E402
from concourse.bass_utils import run_bass_kernel_spmd  # noqa: E402
from concourse.masks import make_identity  # noqa: E402

# ---------------------------------------------------------------- constants
NCORES = 8
S = 2048
SL = S // NCORES  # 256 local rows in phase 1
HID = 4096
Q_LORA = 1536
KV_LORA = 512
ROPE = 64
C = Q_LORA + KV_LORA + ROPE  # 2112
CKV_R = KV_LORA + ROPE  # 576 kv+rope latent cols
NOPE = 128
V_DIM = 128
H = 32
HL = H // NCORES  # 4 local heads
Q_HEAD = NOPE + ROPE  # 192
EPS = 1e-6
NEG = -1e9

F32 = mybir.dt.float32
F16 = mybir.dt.float16
BF16 = mybir.dt.bfloat16

CQ_TILES = Q_LORA // 128  # 12
CKV_TILES = KV_LORA // 128  # 4
HT_TILES = HID // 128  # 32
S_TILES = S // 128  # 16
SQB = 512
NSQB = S // SQB  # 4
EB = 512
NEB = HID // EB  # 8

# phase-1 column blocks, kv+rope first so their collective fires early
CBLOCKS = [(1536, 288), (1824, 288), (0, 512), (512, 512), (1024, 512)]
# feature tiles of the kv+rope latent block: 4x128 (ckv) + 1x64 (rope)
KV_CT = [(0, 128), (128, 128), (256, 128), (384, 128), (512, 64)]  # rel to 1536


# ---------------------------------------------------------------- program
def build_program() -> bass.Bass:
    nc = bacc.Bacc(
        "TRN2",
        target_bir_lowering=False,
        debug=False,
        num_devices=NCORES,
    )

    # hidden is pre-transposed on the host: hidT[ht] = hidden[:, ht*128:...].T
    hid_d = nc.declare_dram_parameter("hid", [HT_TILES, 128, SL], BF16, isOutput=False)
    # w_qkv_a repacked host-side into one contiguous full-width tile per
    # 128-row band: kv+rope cols (576) and q cols (1536) separately, so each
    # DMA moves 1.1-3KB per partition row (the HWDGE rings max out near
    # 110 GB/s on sub-1KB packets).
    wakv_d = nc.declare_dram_parameter("wakv", [HT_TILES, 128, CKV_R], BF16, isOutput=False)
    waq_d = nc.declare_dram_parameter("waq", [HT_TILES, 128, Q_LORA], BF16, isOutput=False)
    wqb_d = nc.declare_dram_parameter("wqb", [Q_LORA, HL * Q_HEAD], BF16, isOutput=False)
    wkvb_d = nc.declare_dram_parameter(
        "wkvb", [KV_LORA, HL * (NOPE + V_DIM)], BF16, isOutput=False
    )
    wo_d = nc.declare_dram_parameter("wo", [HL * V_DIM, HID], BF16, isOutput=False)
    mask_d = nc.declare_dram_parameter("mask", [4, 128, SQB], F32, isOutput=False)
    out_d = nc.declare_dram_parameter("out", [S, HID], BF16, isOutput=True)

    # collective bounce buffers (internal DRAM)
    cc_in_kv = nc.dram_tensor("cc_in_kv", [CKV_R, SL], BF16)
    cc_out_kv = nc.dram_tensor("cc_out_kv", [NCORES, CKV_R, SL], BF16, addr_space="Shared")
    cc_in_q = nc.dram_tensor("cc_in_q", [Q_LORA, SL], BF16)
    cc_out_q = nc.dram_tensor(
        "cc_out_q", [NCORES, Q_LORA, SL], BF16, addr_space="Shared"
    )

    with tile.TileContext(nc, num_cores=NCORES) as tc, ExitStack() as stack:
        # ---------------- small persistent constants
        misc = stack.enter_context(tc.tile_pool(name="misc", bufs=1))
        ident = misc.tile([128, 128], BF16, tag="ident", name="ident")
        make_identity(nc, ident[:])
        ones_sb = misc.tile([128, 1], BF16, tag="ones", name="ones")
        nc.gpsimd.memset(ones_sb[:], 1.0)
        eps_sb = misc.tile([128, 1], F32, tag="eps", name="eps")
        nc.gpsimd.memset(eps_sb[:], EPS)
        onesr_sb = misc.tile([1, 128], F16, tag="onesr", name="onesr")
        nc.gpsimd.memset(onesr_sb[:], 1.0)

        latkv = stack.enter_context(tc.tile_pool(name="latkv", bufs=1))
        latkv_sb = [
            latkv.tile([w, S], BF16, tag=f"latkv{i}", name=f"latkv{i}")
            for i, (_, w) in enumerate(KV_CT)
        ]
        kpeT = latkv_sb[-1]  # [64, S]

        # ---------------- phase 1: a-projection on local rows
        # ht-outer accumulation: each full-width wa tile is streamed exactly
        # once; the per-column-chunk partial sums live in PSUM across the
        # whole pass (kv pass: 4 accumulators, q pass: 6).
        with ExitStack() as p1:
            wa_pool = p1.enter_context(tc.tile_pool(name="wa", bufs=1))
            p1_pool = p1.enter_context(tc.tile_pool(name="p1", bufs=1))
            hidT = [
                p1_pool.tile([128, SL], BF16, tag=f"hidT{ht}", name=f"hidT{ht}")
                for ht in range(HT_TILES)
            ]
            for ht in range(HT_TILES):
                eng = nc.sync if ht % 2 == 0 else nc.scalar
                eng.dma_start(hidT[ht][:], hid_d[ht])

            # wa streams on sync (HWDGE, ~150GB/s) + gpsimd (SW DGE, ~265GB/s).
            # NOT on scalar: DMA-issue instructions with buffer-rotation waits
            # head-of-line block the ACT engine and push the whole RMS ->
            # transpose -> collective chain ~30us late.  gpsimd's odd tiles get
            # fully distinct buffers so no rotation wait can delay the cc_in
            # DMAs queued behind them.
            wakv_sb, waq_sb = [], []
            for ht in range(HT_TILES):
                t = wa_pool.tile([128, CKV_R], BF16, tag=f"wakv{ht % 8}", name=f"wakv{ht}")
                eng = nc.sync if ht % 2 == 0 else nc.gpsimd
                eng.dma_start(t[:], wakv_d[ht])
                wakv_sb.append(t)
            for ht in range(HT_TILES):
                tag = f"waq{ht}" if ht % 2 == 1 else f"waq{ht % 16}"
                t = wa_pool.tile([128, Q_LORA], BF16, tag=tag, name=f"waq{ht}")
                eng = nc.sync if ht % 2 == 0 else nc.gpsimd
                eng.dma_start(t[:], waq_d[ht])
                waq_sb.append(t)

            lat_sb = [
                p1_pool.tile([128, C], BF16, tag=f"lat{s2}", name=f"lat{s2}")
                for s2 in range(2)
            ]
            stat = p1_pool.tile([128, 12], F32, tag="stat", name="stat")
            # local latents^T staging (feature-major, [*, SL])
            latTq_loc = [
                p1_pool.tile([128, SL], BF16, tag=f"latTq{ct}", name=f"latTq{ct}")
                for ct in range(CQ_TILES)
            ]
            latTkv_loc = [
                p1_pool.tile([w, SL], BF16, tag=f"latTkv{i}", name=f"latTkv{i}")
                for i, (_, w) in enumerate(KV_CT)
            ]

            tps_pool = p1.enter_context(tc.tile_pool(name="tps", bufs=2, space="PSUM"))
            psum1 = p1.enter_context(tc.tile_pool(name="psum1", bufs=6, space="PSUM"))

            def a_proj_pass(tiles, chunks, lat_base):
                """lat_sb[:, lat_base+off : +cw] = hidden @ wa_tile_cols for all
                chunks, accumulating over ht with chunk psums held in PSUM."""
                pf = {}
                for ci, (off, cw) in enumerate(chunks):
                    for s2 in range(2):
                        pf[(ci, s2)] = psum1.tile(
                            [128, cw], F32, tag="pf", name=f"pf{lat_base}_{ci}_{s2}"
                        )
                for ht in range(HT_TILES):
                    for s2 in range(2):
                        for ci, (off, cw) in enumerate(chunks):
                            nc.tensor.matmul(
                                pf[(ci, s2)][:],
                                hidT[ht][:, s2 * 128 : (s2 + 1) * 128],
                                tiles[ht][:, off : off + cw],
                                start=(ht == 0),
                                stop=(ht == HT_TILES - 1),
                            )
                for ci, (off, cw) in enumerate(chunks):
                    for s2 in range(2):
                        nc.scalar.copy(
                            lat_sb[s2][:, lat_base + off : lat_base + off + cw],
                            pf[(ci, s2)][:],
                        )

            def transpose_lat(src_col, w, dst):
                """dst[:, s2*128...] = lat_sb[s2][:, src_col:src_col+w]ᵀ"""
                for s2 in range(2):
                    pt = tps_pool.tile([128, 128], BF16, tag="tps", name="tpsl")
                    nc.tensor.transpose(
                        pt[:w, :], lat_sb[s2][:, src_col : src_col + w], ident[:]
                    )
                    nc.vector.tensor_copy(
                        dst[:, s2 * 128 : (s2 + 1) * 128], pt[:w, :]
                    )

            def rms_scale(col0, ncols, stat_base):
                """In-place RMS-normalize lat_sb[:, col0:col0+ncols] (both s2)."""
                nch = ncols // 512
                for s2 in range(2):
                    for ch in range(nch):
                        sq = psum1.tile([128, 512], F32, tag="pf", name=f"sq{s2}_{ch}")
                        nc.scalar.activation(
                            sq[:],
                            lat_sb[s2][:, col0 + ch * 512 : col0 + (ch + 1) * 512],
                            mybir.ActivationFunctionType.Square,
                            accum_out=stat[:, stat_base + ch : stat_base + ch + 1],
                        )
                    for ch in range(1, nch):
                        nc.vector.tensor_add(
                            stat[:, stat_base : stat_base + 1],
                            stat[:, stat_base : stat_base + 1],
                            stat[:, stat_base + ch : stat_base + ch + 1],
                        )
                    nc.scalar.activation(
                        stat[:, stat_base + 4 : stat_base + 5],
                        stat[:, stat_base : stat_base + 1],
                        mybir.ActivationFunctionType.Sqrt,
                        scale=1.0 / ncols,
                        bias=eps_sb[:],
                    )
                    nc.vector.reciprocal(
                        stat[:, stat_base + 5 : stat_base + 6],
                        stat[:, stat_base + 4 : stat_base + 5],
                    )
                    nc.scalar.activation(
                        lat_sb[s2][:, col0 : col0 + ncols],
                        lat_sb[s2][:, col0 : col0 + ncols],
                        mybir.ActivationFunctionType.Copy,
                        scale=stat[:, stat_base + 5 : stat_base + 6],
                    )

            # ---- kv + rope pass first (high priority end to end: nothing may
            # delay the kv collective)
            with tc.high_priority():
                a_proj_pass(wakv_sb, [(0, 288), (288, 288)], Q_LORA)
            # the whole gather-feeding chain runs at high priority: the Tile
            # scheduler must not hoist q-pass matmuls ahead of the transposes
            # or the collective trigger slips by ~40us
            with tc.high_priority():
                rms_scale(Q_LORA, KV_LORA, 0)
                for i, (rel, w) in enumerate(KV_CT):
                    transpose_lat(Q_LORA + rel, w, latTkv_loc[i])
                    nc.gpsimd.dma_start(cc_in_kv[rel : rel + w, :], latTkv_loc[i][:])
                nc.gpsimd.collective_compute(
                    "AllGather",
                    mybir.AluOpType.bypass,
                    replica_groups=[list(range(NCORES))],
                    ins=[cc_in_kv[:].opt()],
                    outs=[cc_out_kv[:].opt()],
                )
                # gathered kv latents -> SBUF on the (otherwise idle) gpsimd q
                cc_kv_view = cc_out_kv[:].rearrange("j c s -> c j s")
                for i, (rel, w) in enumerate(KV_CT):
                    nc.gpsimd.dma_start(
                        latkv_sb[i][:].rearrange("c (j s) -> c j s", j=NCORES),
                        cc_kv_view[rel : rel + w],
                    )

            # ---- q pass
            a_proj_pass(waq_sb, [(0, 512), (512, 512), (1024, 512)], 0)
            with tc.high_priority():
                rms_scale(0, Q_LORA, 6)
                for ct in range(CQ_TILES):
                    transpose_lat(ct * 128, 128, latTq_loc[ct])
                    nc.gpsimd.dma_start(
                        cc_in_q[ct * 128 : (ct + 1) * 128, :], latTq_loc[ct][:]
                    )
                nc.gpsimd.collective_compute(
                    "AllGather",
                    mybir.AluOpType.bypass,
                    replica_groups=[list(range(NCORES))],
                    ins=[cc_in_q[:].opt()],
                    outs=[cc_out_q[:].opt()],
                )

        # phase-2 up-proj weights: loaded on the HWDGE queues right behind the
        # wa stream (they arrive long before first use)
        wkvb_pool = stack.enter_context(tc.tile_pool(name="wkvb", bufs=1))
        wkvb_sb = [
            wkvb_pool.tile(
                [128, HL * (NOPE + V_DIM)], BF16, tag=f"wkvb{kt}", name=f"wkvb{kt}"
            )
            for kt in range(CKV_TILES)
        ]
        for kt in range(CKV_TILES):
            nc.sync.dma_start(wkvb_sb[kt][:], wkvb_d[kt * 128 : (kt + 1) * 128, :])
        wqb_pool = stack.enter_context(tc.tile_pool(name="wqb", bufs=1))
        wqb_sb = [
            wqb_pool.tile([128, HL * Q_HEAD], BF16, tag=f"wqb{kt}", name=f"wqb{kt}")
            for kt in range(CQ_TILES)
        ]
        for kt in range(CQ_TILES):
            nc.scalar.dma_start(wqb_sb[kt][:], wqb_d[kt * 128 : (kt + 1) * 128, :])

        # ---------------- phase 2a: q up-projection (head-sharded)
        # wqb is host-reordered: cols [h0 nope | h1 nope | h2 nope | h3 nope |
        # rope h0h1 | rope h2h3], so all 6 column groups are 128 wide.
        qT = stack.enter_context(tc.tile_pool(name="qT", bufs=1))
        qTA = [qT.tile([128, S], BF16, tag=f"qTA{h}", name=f"qTA{h}") for h in range(HL)]
        qTB = [qT.tile([64, S], BF16, tag=f"qTB{h}", name=f"qTB{h}") for h in range(HL)]
        with ExitStack() as p2q:
            latq = p2q.enter_context(tc.tile_pool(name="latq", bufs=1))
            latq_sb = [
                latq.tile([128, S], BF16, tag=f"latq{ct}", name=f"latq{ct}")
                for ct in range(CQ_TILES)
            ]
            cc_q_view = cc_out_q[:].rearrange("j c s -> c j s")
            for ct in range(CQ_TILES):
                eng = (nc.sync, nc.scalar, nc.gpsimd)[ct % 3]
                eng.dma_start(
                    latq_sb[ct][:].rearrange("c (j s) -> c j s", j=NCORES),
                    cc_q_view[ct * 128 : (ct + 1) * 128],
                )
            pq_pool = p2q.enter_context(tc.tile_pool(name="pq", bufs=8, space="PSUM"))
            for part in range(6):
                col0 = part * 128
                pqs = [
                    pq_pool.tile([128, SQB], F32, tag="pq", name=f"pq{sqb}")
                    for sqb in range(NSQB)
                ]
                for kt in range(CQ_TILES):
                    for sqb in range(NSQB):
                        nc.tensor.matmul(
                            pqs[sqb][:],
                            wqb_sb[kt][:, col0 : col0 + 128],
                            latq_sb[kt][:, sqb * SQB : (sqb + 1) * SQB],
                            start=(kt == 0),
                            stop=(kt == CQ_TILES - 1),
                        )
                for sqb in range(NSQB):
                    sl = slice(sqb * SQB, (sqb + 1) * SQB)
                    if part < 4:
                        nc.scalar.copy(qTA[part][:, sl], pqs[sqb][:])
                    else:
                        h0 = (part - 4) * 2
                        nc.scalar.copy(qTB[h0][:, sl], pqs[sqb][0:64, :])
                        nc.scalar.copy(qTB[h0 + 1][:, sl], pqs[sqb][64:128, :])

        # ---------------- phase 2b: k_nope^T and V up-projections
        kvpool = stack.enter_context(tc.tile_pool(name="kvpool", bufs=1))
        knopeT = [
            kvpool.tile([128, S], BF16, tag=f"knopeT{h}", name=f"knopeT{h}")
            for h in range(HL)
        ]
        v_sb = [
            kvpool.tile([128, HL * V_DIM], BF16, tag=f"v{st}", name=f"v{st}")
            for st in range(S_TILES)
        ]
        with ExitStack() as p2kv:
            pkv_pool = p2kv.enter_context(tc.tile_pool(name="pkv", bufs=4, space="PSUM"))
            for h in range(HL):
                for skb in range(NSQB):
                    pk = pkv_pool.tile([128, SQB], F32, tag="pkv", name="pk")
                    for kt in range(CKV_TILES):
                        nc.tensor.matmul(
                            pk[:],
                            wkvb_sb[kt][
                                :, h * (NOPE + V_DIM) : h * (NOPE + V_DIM) + NOPE
                            ],
                            latkv_sb[kt][:, skb * SQB : (skb + 1) * SQB],
                            start=(kt == 0),
                            stop=(kt == CKV_TILES - 1),
                        )
                    nc.scalar.copy(knopeT[h][:, skb * SQB : (skb + 1) * SQB], pk[:])
            for st in range(S_TILES):
                pv = pkv_pool.tile([128, HL * V_DIM], F32, tag="pkv", name="pv")
                for kt in range(CKV_TILES):
                    rhs = wkvb_sb[kt][:].rearrange("c (h d) -> c h d", h=HL)[:, :, NOPE:]
                    nc.tensor.matmul(
                        pv[:],
                        latkv_sb[kt][:, st * 128 : (st + 1) * 128],
                        rhs,
                        start=(kt == 0),
                        stop=(kt == CKV_TILES - 1),
                    )
                nc.scalar.copy(v_sb[st][:], pv[:])

        outT_pool = stack.enter_context(tc.tile_pool(name="outT", bufs=1))
        outT = [
            outT_pool.tile([128, S], BF16, tag=f"outT{h}", name=f"outT{h}")
            for h in range(HL)
        ]
        wo_pool = stack.enter_context(tc.tile_pool(name="wo", bufs=1))
        wo_sb = [
            wo_pool.tile([128, HID], BF16, tag=f"wo{h}", name=f"wo{h}")
            for h in range(HL)
        ]
        for h in range(HL):
            nc.scalar.dma_start(wo_sb[h][:], wo_d[h * 128 : (h + 1) * 128, :])

        # ---------------- attention (causal, block-skipped, column-sliced)
        # Per score tile: nope+rope matmuls and the AV matmul stream only the
        # live (causal) column range.  exp tiles are summed in groups of 4 on
        # the DVE; one ones-matmul per group accumulates the softmax
        # denominator.  AV matmuls are deferred 2 tiles so the PE never waits
        # on the DVE-mask -> ACT-exp chain; the renormalization (fp16
        # reciprocal broadcast) is deferred by one (head, sq-block) pair.
        with ExitStack() as p2a:
            am_pool = p2a.enter_context(tc.tile_pool(name="am", bufs=1))
            mask_sb = am_pool.tile([128, 4 * SQB], F32, tag="mask", name="mask")
            for d in range(4):
                nc.gpsimd.dma_start(mask_sb[:, d * SQB : (d + 1) * SQB], mask_d[d])

            ps_pool = p2a.enter_context(tc.tile_pool(name="ps", bufs=4, space="PSUM"))
            psum_sum_pool = p2a.enter_context(
                tc.tile_pool(name="psums", bufs=2, space="PSUM")
            )
            psum_o_pool = p2a.enter_context(
                tc.tile_pool(name="psumo", bufs=2, space="PSUM")
            )
            a_pool = p2a.enter_context(tc.tile_pool(name="apool", bufs=8))
            sg_pool = p2a.enter_context(tc.tile_pool(name="sgpool", bufs=3))
            bc_pool = p2a.enter_context(tc.tile_pool(name="bcpool", bufs=2))

            av_q = []  # score tiles awaiting their AV matmul
            rs_q = []  # groups awaiting the denominator ones-matmul
            ep_q = []  # pairs awaiting the renormalization epilogue

            def drain_av():
                a, h, tk, nk, po, lv = av_q.pop(0)
                nc.tensor.matmul(
                    po[:, lv:],
                    v_sb[tk][:, h * V_DIM : (h + 1) * V_DIM],
                    a[:, lv:],
                    start=(tk == 0),
                    stop=(tk == nk - 1),
                )

            def drain_rs():
                s_g, h, bq, g, psum, po = rs_q.pop(0)
                nc.tensor.matmul(
                    psum[:], ones_sb[:], s_g[:], start=(g == 0), stop=(g == bq)
                )
                if g == bq:
                    # copy the denominators out of PSUM first (fast, frees the
                    # bank) -- the [1,512] reciprocal runs 2.7us on one DVE lane
                    sum_sb = bc_pool.tile([1, SQB], F32, tag="sums", name="sums")
                    nc.scalar.copy(sum_sb[:], psum[:])
                    rs = bc_pool.tile([1, SQB], F16, tag="rs", name="rs")
                    with nc.allow_low_precision(
                        reason="softmax denom reciprocal broadcast in fp16"
                    ):
                        nc.vector.reciprocal(rs[:], sum_sb[:])
                    ep_q.append((h, bq, po, rs))

            def drain_epilogue():
                h, bq, po, rs = ep_q.pop(0)
                bc_ps = ps_pool.tile([128, SQB], F32, tag="ps", name="bc_ps")
                nc.tensor.matmul(bc_ps[:], onesr_sb[:], rs[:], start=True, stop=True)
                bc_sb = bc_pool.tile([128, SQB], F32, tag="bc", name="bc_sb")
                nc.scalar.copy(bc_sb[:], bc_ps[:])
                nc.vector.tensor_mul(
                    outT[h][:, bq * SQB : (bq + 1) * SQB], po[:], bc_sb[:]
                )

            for h in range(HL):
                for bq in range(NSQB):
                    nk = 4 * (bq + 1)
                    psum = psum_sum_pool.tile([1, SQB], F32, tag="psums", name="psum")
                    po = psum_o_pool.tile([128, SQB], F32, tag="psumo", name="po")
                    a_grp = []
                    s_g = None
                    for tk in range(nk):
                        d = tk - 4 * bq
                        lv = 128 * d if d > 0 else 0
                        ps = ps_pool.tile([128, SQB], F32, tag="ps", name="ps")
                        nc.tensor.matmul(
                            ps[:, lv:],
                            knopeT[h][:, tk * 128 : (tk + 1) * 128],
                            qTA[h][:, bq * SQB + lv : (bq + 1) * SQB],
                            start=True,
                            stop=False,
                        )
                        nc.tensor.matmul(
                            ps[:, lv:],
                            kpeT[:, tk * 128 : (tk + 1) * 128],
                            qTB[h][:, bq * SQB + lv : (bq + 1) * SQB],
                            start=False,
                            stop=True,
                        )
                        if d >= 0:
                            nc.vector.tensor_add(
                                ps[:, lv:],
                                ps[:, lv:],
                                mask_sb[:, d * SQB + lv : (d + 1) * SQB],
                            )
                        a = a_pool.tile([128, SQB], BF16, tag="a", name="a")
                        nc.scalar.activation(
                            a[:, lv:], ps[:, lv:], mybir.ActivationFunctionType.Exp
                        )
                        # ---- DVE group sum for the softmax denominator
                        r = tk % 4
                        if r == 0:
                            a_grp = [(a, lv)]
                        else:
                            a_grp.append((a, lv))
                            if r == 1:
                                s_g = sg_pool.tile(
                                    [128, SQB], BF16, tag="sg", name="sg"
                                )
                                a0, lv0 = a_grp[0]
                                nc.vector.tensor_add(
                                    s_g[:, lv:], a0[:, lv:], a[:, lv:]
                                )
                                if lv > lv0:
                                    nc.vector.tensor_copy(
                                        s_g[:, lv0:lv], a0[:, lv0:lv]
                                    )
                            else:
                                nc.vector.tensor_add(
                                    s_g[:, lv:], s_g[:, lv:], a[:, lv:]
                                )
                        av_q.append((a, h, tk, nk, po, lv))
                        if r == 3:
                            rs_q.append((s_g, h, bq, tk // 4, psum, po))
                        while len(av_q) > 2:
                            drain_av()
                        while len(rs_q) > 1:
                            drain_rs()
                        while len(ep_q) > 1:
                            drain_epilogue()
            while av_q:
                drain_av()
            while rs_q:
                drain_rs()
            while ep_q:
                drain_epilogue()

        # ---------------- o_proj (partial: summed across cores on host)
        with ExitStack() as p2o:
            pe_pool = p2o.enter_context(tc.tile_pool(name="pe", bufs=4, space="PSUM"))
            stage_pool = p2o.enter_context(tc.tile_pool(name="stage", bufs=3))
            for st in range(S_TILES):
                for half in range(2):
                    stg = stage_pool.tile([128, 4 * EB], BF16, tag="stage", name="stg")
                    for ebl in range(4):
                        eb = half * 4 + ebl
                        pe = pe_pool.tile([128, EB], F32, tag="pe", name="pe")
                        for h in range(HL):
                            nc.tensor.matmul(
                                pe[:],
                                outT[h][:, st * 128 : (st + 1) * 128],
                                wo_sb[h][:, eb * EB : (eb + 1) * EB],
                                start=(h == 0),
                                stop=(h == HL - 1),
                            )
                        nc.vector.tensor_copy(
                            stg[:, ebl * EB : (ebl + 1) * EB], pe[:]
                        )
                    nc.gpsimd.dma_start(
                        out_d[
                            st * 128 : (st + 1) * 128,
                            half * 4 * EB : (half + 1) * 4 * EB,
                        ],
                        stg[:],
                    )

    nc.compile()
    return nc


_PROGRAM_CACHE = {}


def _get_program() -> bass.Bass:
    if "nc" not in _PROGRAM_CACHE:
        _PROGRAM_CACHE["nc"] = build_program()
    return _PROGRAM_CACHE["nc"]


def _make_mask() -> np.ndarray:
    # mask[d, p, f] for diagonal score tiles: sk-tile tk = 4*bq + d.
    # valid (sq >= sk)  <=>  f >= 128*d + p
    d = np.arange(4)[:, None, None]
    p = np.arange(128)[None, :, None]
    f = np.arange(SQB)[None, None, :]
    return np.where(f >= 128 * d + p, 0.0, NEG).astype(np.float32)


def prepare_inputs(
    hidden_states, w_qkv_a, q_a_gamma, w_q_b, kv_a_gamma, w_kv_b, w_o, b_o
):
    """Host-side prep: fold gammas + attention scale into B weights, cast to
    bf16, repack wa into contiguous tiles, slice per core."""
    bf = ml_dtypes.bfloat16
    hs = np.asarray(hidden_states, np.float32).reshape(S, HID)
    scale = float(Q_HEAD) ** -0.5
    wqb_eff = (
        np.asarray(w_q_b, np.float32)
        * np.asarray(q_a_gamma, np.float32)[:, None]
        * scale
    )
    wkvb_eff = (
        np.asarray(w_kv_b, np.float32) * np.asarray(kv_a_gamma, np.float32)[:, None]
    )
    wa_bf = np.asarray(w_qkv_a, np.float32).astype(bf)
    hs_bf = hs.astype(bf)
    mask = _make_mask()

    # pack wa into one contiguous full-width tile per 128-row band
    wa3 = np.ascontiguousarray(wa_bf.reshape(HT_TILES, 128, C))
    wakv = np.ascontiguousarray(wa3[:, :, Q_LORA:])
    waq = np.ascontiguousarray(wa3[:, :, :Q_LORA])

    wqb_r = wqb_eff.reshape(Q_LORA, H, Q_HEAD)
    wkvb_r = wkvb_eff.reshape(KV_LORA, H, NOPE + V_DIM)
    wo_r = np.asarray(w_o, np.float32).reshape(H, V_DIM, HID)

    in_maps = []
    for c in range(NCORES):
        hsl = np.ascontiguousarray(
            hs_bf[c * SL : (c + 1) * SL]
            .reshape(SL, HT_TILES, 128)
            .transpose(1, 2, 0)
        )
        wqb_h = wqb_r[:, c * HL : (c + 1) * HL]  # [Q_LORA, 4, 192]
        # column order: h0..h3 nope (128 each), rope pairs (h0h1, h2h3)
        wqb_c = np.ascontiguousarray(
            np.concatenate(
                [
                    wqb_h[:, 0, :NOPE],
                    wqb_h[:, 1, :NOPE],
                    wqb_h[:, 2, :NOPE],
                    wqb_h[:, 3, :NOPE],
                    wqb_h[:, 0, NOPE:],
                    wqb_h[:, 1, NOPE:],
                    wqb_h[:, 2, NOPE:],
                    wqb_h[:, 3, NOPE:],
                ],
                axis=1,
            ).astype(bf)
        )
        wkvb_c = np.ascontiguousarray(
            wkvb_r[:, c * HL : (c + 1) * HL]
            .reshape(KV_LORA, HL * (NOPE + V_DIM))
            .astype(bf)
        )
        wo_c = np.ascontiguousarray(
            wo_r[c * HL : (c + 1) * HL].reshape(HL * V_DIM, HID).astype(bf)
        )
        in_maps.append(
            {
                "hid": hsl,
                "wakv": wakv,
                "waq": waq,
                "wqb": wqb_c,
                "wkvb": wkvb_c,
                "wo": wo_c,
                "mask": mask,
            }
        )
    return in_maps


def kernel(**inputs) -> np.ndarray:
    in_maps = prepare_inputs(**inputs)
    nc = _get_program()
    res = run_bass_kernel_spmd(nc, in_maps, list(range(NCORES)))
    out = np.zeros((S, HID), np.float64)
    for r in res.results:
        out += np.asarray(r["out"], np.float32)
    out = out.astype(np.float32) + np.asarray(inputs["b_o"], np.float32)[None, :]
    return out.reshape(1, S, HID)


# revision 32
# speedup vs baseline: 1.0344x; 1.0344x over previous
"""
MLA attention (DeepSeek-style) on 8 TRN2 NeuronCores.

Sharding:
  phase 1 (LoRA-A projection + RMSNorm): sharded over sequence (256 rows/core),
    result transposed to feature-major and AllGathered (bf16 latents).
    The kv+rope latent columns are computed first and gathered in an early
    collective that overlaps the rest of phase 1; the q latents follow.
  phase 2 (q/kv up-proj, attention, o_proj): sharded over heads (4 heads/core),
    w_o input-dim sharded; partial outputs summed on the host (the all-reduce).

All heavy matmuls run in bf16 with fp32 PSUM accumulation.
Everything feature-major ("X^T" layout [feature, seq]) in phase 2 so no big
transposes are needed.

Perf notes vs the first working version:
  - w_qkv_a is repacked on the host into contiguous [128, cw] tiles and
    streamed on BOTH HWDGE queues (sync + scalar); the strided layout capped
    the single queue at ~150 GB/s and starved the PE through phase 1.
  - collective input DMAs + triggers go through the gpsimd queue so they are
    not stuck behind the weight stream; q-latent readback is split across the
    two HWDGE queues which are idle by then.
  - w_q_b is column-reordered on the host (per-head nope blocks, then
    rope-pairs of two heads) so every up-proj matmul uses the full 128-wide
    PE array.
  - softmax denominators: exp tiles are pre-summed in groups of 4 on the DVE,
    one ones-matmul per group instead of per tile (4x fewer PE columns).
  - causal diagonal tiles only stream their live column range.
  - the reciprocal-broadcast matmul runs in fp16 (it was silently fp32 =
    4 cycles/col); the renormalization multiply reads both PSUM operands
    directly via scalar_tensor_tensor.
  - o_proj partials are written out in bf16 (host sums in fp64).
"""

import os
import sys
from contextlib import ExitStack

import numpy as np

for _p in ("/opt/trn_rl_repo", "/root/.axon_site/_ro/trn_rl_repo"):
    if os.path.isdir(_p) and _p not in sys.path:
        sys.path.insert(0, _p)

import ml_dtypes  # noqa: E402

import concourse.bacc as bacc  # noqa: E402
import concourse.bass as bass  # noqa: E402
import concourse.mybir as mybir  # noqa: E402
import concourse.tile as tile  # noqa: E402
from concourse.bass_utils import run_bass_kernel_spmd  # noqa: E402
from concourse.masks import make_identity  # noqa: E402

# ---------------------------------------------------------------- constants
NCORES = 8
S = 2048
SL = S // NCORES  # 256 local rows in phase 1
HID = 4096
Q_LORA = 1536
KV_LORA = 512
ROPE = 64
C = Q_LORA + KV_LORA + ROPE  # 2112
CKV_R = KV_LORA + ROPE  # 576 kv+rope latent cols
NOPE = 128
V_DIM = 128
H = 32
HL = H // NCORES  # 4 local heads
Q_HEAD = NOPE + ROPE  # 192
EPS = 1e-6
NEG = -1e9

F32 = mybir.dt.float32
F16 = mybir.dt.float16
BF16 = mybir.dt.bfloat16

CQ_TILES = Q_LORA // 128  # 12
CKV_TILES = KV_LORA // 128  # 4
HT_TILES = HID // 128  # 32
S_TILES = S // 128  # 16
SQB = 512
NSQB = S // SQB  # 4
EB = 512
NEB = HID // EB  # 8

# phase-1 column blocks, kv+rope first so their collective fires early
CBLOCKS = [(1536, 288), (1824, 288), (0, 512), (512, 512), (1024, 512)]
# feature tiles of the kv+rope latent block: 4x128 (ckv) + 1x64 (rope)
KV_CT = [(0, 128), (128, 128), (256, 128), (384, 128), (512, 64)]  # rel to 1536


# ---------------------------------------------------------------- program
def build_program() -> bass.Bass:
    nc = bacc.Bacc(
        "TRN2",
        target_bir_lowering=False,
        debug=False,
        num_devices=NCORES,
    )

    # hidden pre-transposed AND partition-interleaved on the host:
    # hid[p, ht*SL + s] = hidden[s, ht*128 + p] -> one DMA with 16KB rows
    hid_d = nc.declare_dram_parameter("hid", [128, HT_TILES * SL], BF16, isOutput=False)
    # w_qkv_a repacked host-side into one contiguous full-width tile per
    # 128-row band: kv+rope cols (576) and q cols (1536) separately, so each
    # DMA moves 1.1-3KB per partition row (the HWDGE rings max out near
    # 110 GB/s on sub-1KB packets).
    wakv_d = nc.declare_dram_parameter("wakv", [HT_TILES, 128, CKV_R], BF16, isOutput=False)
    waq_d = nc.declare_dram_parameter("waq", [HT_TILES, 128, Q_LORA], BF16, isOutput=False)
    wqb_d = nc.declare_dram_parameter("wqb", [Q_LORA, HL * Q_HEAD], BF16, isOutput=False)
    wkvb_d = nc.declare_dram_parameter(
        "wkvb", [KV_LORA, HL * (NOPE + V_DIM)], BF16, isOutput=False
    )
    wo_d = nc.declare_dram_parameter("wo", [HL * V_DIM, HID], BF16, isOutput=False)
    mask_d = nc.declare_dram_parameter("mask", [4, 128, SQB], F32, isOutput=False)
    out_d = nc.declare_dram_parameter("out", [S, HID], BF16, isOutput=True)

    # collective bounce buffers (internal DRAM)
    cc_in_kv = nc.dram_tensor("cc_in_kv", [CKV_R, SL], BF16)
    cc_out_kv = nc.dram_tensor("cc_out_kv", [NCORES, CKV_R, SL], BF16, addr_space="Shared")
    cc_in_q = nc.dram_tensor("cc_in_q", [Q_LORA, SL], BF16)
    cc_out_q = nc.dram_tensor(
        "cc_out_q", [NCORES, Q_LORA, SL], BF16, addr_space="Shared"
    )

    with tile.TileContext(nc, num_cores=NCORES) as tc, ExitStack() as stack:
        # ---------------- small persistent constants
        misc = stack.enter_context(tc.tile_pool(name="misc", bufs=1))
        ident = misc.tile([128, 128], BF16, tag="ident", name="ident")
        make_identity(nc, ident[:])
        ones_sb = misc.tile([128, 1], BF16, tag="ones", name="ones")
        nc.gpsimd.memset(ones_sb[:], 1.0)
        eps_sb = misc.tile([128, 1], F32, tag="eps", name="eps")
        nc.gpsimd.memset(eps_sb[:], EPS)
        onesr_sb = misc.tile([1, 128], F16, tag="onesr", name="onesr")
        nc.gpsimd.memset(onesr_sb[:], 1.0)

        latkv = stack.enter_context(tc.tile_pool(name="latkv", bufs=1))
        latkv_sb = [
            latkv.tile([w, S], BF16, tag=f"latkv{i}", name=f"latkv{i}")
            for i, (_, w) in enumerate(KV_CT)
        ]
        kpeT = latkv_sb[-1]  # [64, S]

        # ---------------- phase 1: a-projection on local rows
        # ht-outer accumulation: each full-width wa tile is streamed exactly
        # once; the per-column-chunk partial sums live in PSUM across the
        # whole pass (kv pass: 4 accumulators, q pass: 6).
        with ExitStack() as p1:
            wa_pool = p1.enter_context(tc.tile_pool(name="wa", bufs=1))
            p1_pool = p1.enter_context(tc.tile_pool(name="p1", bufs=1))
            hidT_all = p1_pool.tile(
                [128, HT_TILES * SL], BF16, tag="hidT", name="hidT"
            )
            nc.gpsimd.dma_start(hidT_all[:], hid_d[:])

            def hidT(ht, s2):
                return hidT_all[:, ht * SL + s2 * 128 : ht * SL + (s2 + 1) * 128]

            # wa streams on sync (HWDGE, ~150GB/s) + gpsimd (SW DGE, ~265GB/s).
            # NOT on scalar: DMA-issue instructions with buffer-rotation waits
            # head-of-line block the ACT engine and push the whole RMS ->
            # transpose -> collective chain ~30us late.  gpsimd's odd tiles get
            # fully distinct buffers so no rotation wait can delay the cc_in
            # DMAs queued behind them.
            wakv_sb, waq_sb = [], []
            for ht in range(HT_TILES):
                t = wa_pool.tile([128, CKV_R], BF16, tag=f"wakv{ht % 8}", name=f"wakv{ht}")
                eng = nc.sync if ht % 2 == 0 else nc.gpsimd
                eng.dma_start(t[:], wakv_d[ht])
                wakv_sb.append(t)
            for ht in range(HT_TILES):
                tag = f"waq{ht}" if ht % 2 == 1 else f"waq{ht % 16}"
                t = wa_pool.tile([128, Q_LORA], BF16, tag=tag, name=f"waq{ht}")
                eng = nc.sync if ht % 2 == 0 else nc.gpsimd
                eng.dma_start(t[:], waq_d[ht])
                waq_sb.append(t)

            lat_sb = [
                p1_pool.tile([128, C], BF16, tag=f"lat{s2}", name=f"lat{s2}")
                for s2 in range(2)
            ]
            stat = p1_pool.tile([128, 12], F32, tag="stat", name="stat")
            # local latents^T staging (feature-major, [*, SL])
            latTq_loc = [
                p1_pool.tile([128, SL], BF16, tag=f"latTq{ct}", name=f"latTq{ct}")
                for ct in range(CQ_TILES)
            ]
            latTkv_loc = [
                p1_pool.tile([w, SL], BF16, tag=f"latTkv{i}", name=f"latTkv{i}")
                for i, (_, w) in enumerate(KV_CT)
            ]

            tps_pool = p1.enter_context(tc.tile_pool(name="tps", bufs=2, space="PSUM"))
            psum1 = p1.enter_context(tc.tile_pool(name="psum1", bufs=6, space="PSUM"))

            def a_proj_pass(tiles, chunks, lat_base):
                """lat_sb[:, lat_base+off : +cw] = hidden @ wa_tile_cols for all
                chunks, accumulating over ht with chunk psums held in PSUM."""
                pf = {}
                for ci, (off, cw) in enumerate(chunks):
                    for s2 in range(2):
                        pf[(ci, s2)] = psum1.tile(
                            [128, cw], F32, tag="pf", name=f"pf{lat_base}_{ci}_{s2}"
                        )
                for ht in range(HT_TILES):
                    for s2 in range(2):
                        for ci, (off, cw) in enumerate(chunks):
                            nc.tensor.matmul(
                                pf[(ci, s2)][:],
                                hidT(ht, s2),
                                tiles[ht][:, off : off + cw],
                                start=(ht == 0),
                                stop=(ht == HT_TILES - 1),
                            )
                for ci, (off, cw) in enumerate(chunks):
                    for s2 in range(2):
                        nc.scalar.copy(
                            lat_sb[s2][:, lat_base + off : lat_base + off + cw],
                            pf[(ci, s2)][:],
                        )

            def transpose_lat(src_col, w, dst):
                """dst[:, s2*128...] = lat_sb[s2][:, src_col:src_col+w]ᵀ"""
                for s2 in range(2):
                    pt = tps_pool.tile([128, 128], BF16, tag="tps", name="tpsl")
                    nc.tensor.transpose(
                        pt[:w, :], lat_sb[s2][:, src_col : src_col + w], ident[:]
                    )
                    nc.vector.tensor_copy(
                        dst[:, s2 * 128 : (s2 + 1) * 128], pt[:w, :]
                    )

            def rms_scale(col0, ncols, stat_base):
                """In-place RMS-normalize lat_sb[:, col0:col0+ncols] (both s2)."""
                nch = ncols // 512
                for s2 in range(2):
                    for ch in range(nch):
                        sq = psum1.tile([128, 512], F32, tag="pf", name=f"sq{s2}_{ch}")
                        nc.scalar.activation(
                            sq[:],
                            lat_sb[s2][:, col0 + ch * 512 : col0 + (ch + 1) * 512],
                            mybir.ActivationFunctionType.Square,
                            accum_out=stat[:, stat_base + ch : stat_base + ch + 1],
                        )
                    for ch in range(1, nch):
                        nc.vector.tensor_add(
                            stat[:, stat_base : stat_base + 1],
                            stat[:, stat_base : stat_base + 1],
                            stat[:, stat_base + ch : stat_base + ch + 1],
                        )
                    nc.scalar.activation(
                        stat[:, stat_base + 4 : stat_base + 5],
                        stat[:, stat_base : stat_base + 1],
                        mybir.ActivationFunctionType.Sqrt,
                        scale=1.0 / ncols,
                        bias=eps_sb[:],
                    )
                    nc.vector.reciprocal(
                        stat[:, stat_base + 5 : stat_base + 6],
                        stat[:, stat_base + 4 : stat_base + 5],
                    )
                    nc.scalar.activation(
                        lat_sb[s2][:, col0 : col0 + ncols],
                        lat_sb[s2][:, col0 : col0 + ncols],
                        mybir.ActivationFunctionType.Copy,
                        scale=stat[:, stat_base + 5 : stat_base + 6],
                    )

            # ---- kv + rope pass first (high priority end to end: nothing may
            # delay the kv collective)
            with tc.high_priority():
                a_proj_pass(wakv_sb, [(0, 288), (288, 288)], Q_LORA)
            # the whole gather-feeding chain runs at high priority: the Tile
            # scheduler must not hoist q-pass matmuls ahead of the transposes
            # or the collective trigger slips by ~40us
            with tc.high_priority():
                rms_scale(Q_LORA, KV_LORA, 0)
                for i, (rel, w) in enumerate(KV_CT):
                    transpose_lat(Q_LORA + rel, w, latTkv_loc[i])
                    nc.gpsimd.dma_start(cc_in_kv[rel : rel + w, :], latTkv_loc[i][:])
                nc.gpsimd.collective_compute(
                    "AllGather",
                    mybir.AluOpType.bypass,
                    replica_groups=[list(range(NCORES))],
                    ins=[cc_in_kv[:].opt()],
                    outs=[cc_out_kv[:].opt()],
                )
                # gathered kv latents -> SBUF via sync (it is drained by now;
                # on gpsimd this wait would delay the q collective trigger)
                cc_kv_view = cc_out_kv[:].rearrange("j c s -> c j s")
                for i, (rel, w) in enumerate(KV_CT):
                    nc.sync.dma_start(
                        latkv_sb[i][:].rearrange("c (j s) -> c j s", j=NCORES),
                        cc_kv_view[rel : rel + w],
                    )

            # ---- q pass
            a_proj_pass(waq_sb, [(0, 512), (512, 512), (1024, 512)], 0)
            with tc.high_priority():
                rms_scale(0, Q_LORA, 6)
                for ct in range(CQ_TILES):
                    transpose_lat(ct * 128, 128, latTq_loc[ct])
                    nc.gpsimd.dma_start(
                        cc_in_q[ct * 128 : (ct + 1) * 128, :], latTq_loc[ct][:]
                    )
                nc.gpsimd.collective_compute(
                    "AllGather",
                    mybir.AluOpType.bypass,
                    replica_groups=[list(range(NCORES))],
                    ins=[cc_in_q[:].opt()],
                    outs=[cc_out_q[:].opt()],
                )

        # phase-2 up-proj weights: loaded on the HWDGE queues right behind the
        # wa stream (they arrive long before first use)
        wkvb_pool = stack.enter_context(tc.tile_pool(name="wkvb", bufs=1))
        wkvb_sb = [
            wkvb_pool.tile(
                [128, HL * (NOPE + V_DIM)], BF16, tag=f"wkvb{kt}", name=f"wkvb{kt}"
            )
            for kt in range(CKV_TILES)
        ]
        for kt in range(CKV_TILES):
            nc.sync.dma_start(wkvb_sb[kt][:], wkvb_d[kt * 128 : (kt + 1) * 128, :])
        wqb_pool = stack.enter_context(tc.tile_pool(name="wqb", bufs=1))
        wqb_sb = [
            wqb_pool.tile([128, HL * Q_HEAD], BF16, tag=f"wqb{kt}", name=f"wqb{kt}")
            for kt in range(CQ_TILES)
        ]
        for kt in range(CQ_TILES):
            nc.scalar.dma_start(wqb_sb[kt][:], wqb_d[kt * 128 : (kt + 1) * 128, :])

        # ---------------- phase 2a: q up-projection (head-sharded)
        # wqb is host-reordered: cols [h0 nope | h1 nope | h2 nope | h3 nope |
        # rope h0h1 | rope h2h3], so all 6 column groups are 128 wide.
        qT = stack.enter_context(tc.tile_pool(name="qT", bufs=1))
        qTA = [qT.tile([128, S], BF16, tag=f"qTA{h}", name=f"qTA{h}") for h in range(HL)]
        qTB = [qT.tile([64, S], BF16, tag=f"qTB{h}", name=f"qTB{h}") for h in range(HL)]
        with ExitStack() as p2q:
            latq = p2q.enter_context(tc.tile_pool(name="latq", bufs=1))
            latq_sb = [
                latq.tile([128, S], BF16, tag=f"latq{ct}", name=f"latq{ct}")
                for ct in range(CQ_TILES)
            ]
            cc_q_view = cc_out_q[:].rearrange("j c s -> c j s")
            for ct in range(CQ_TILES):
                eng = (nc.sync, nc.scalar, nc.gpsimd)[ct % 3]
                eng.dma_start(
                    latq_sb[ct][:].rearrange("c (j s) -> c j s", j=NCORES),
                    cc_q_view[ct * 128 : (ct + 1) * 128],
                )
            pq_pool = p2q.enter_context(tc.tile_pool(name="pq", bufs=8, space="PSUM"))
            for part in range(6):
                col0 = part * 128
                pqs = [
                    pq_pool.tile([128, SQB], F32, tag="pq", name=f"pq{sqb}")
                    for sqb in range(NSQB)
                ]
                for kt in range(CQ_TILES):
                    for sqb in range(NSQB):
                        nc.tensor.matmul(
                            pqs[sqb][:],
                            wqb_sb[kt][:, col0 : col0 + 128],
                            latq_sb[kt][:, sqb * SQB : (sqb + 1) * SQB],
                            start=(kt == 0),
                            stop=(kt == CQ_TILES - 1),
                        )
                for sqb in range(NSQB):
                    sl = slice(sqb * SQB, (sqb + 1) * SQB)
                    if part < 4:
                        nc.scalar.copy(qTA[part][:, sl], pqs[sqb][:])
                    else:
                        h0 = (part - 4) * 2
                        nc.scalar.copy(qTB[h0][:, sl], pqs[sqb][0:64, :])
                        nc.scalar.copy(qTB[h0 + 1][:, sl], pqs[sqb][64:128, :])

        # ---------------- phase 2b: k_nope^T and V up-projections
        kvpool = stack.enter_context(tc.tile_pool(name="kvpool", bufs=1))
        knopeT = [
            kvpool.tile([128, S], BF16, tag=f"knopeT{h}", name=f"knopeT{h}")
            for h in range(HL)
        ]
        v_sb = [
            kvpool.tile([128, HL * V_DIM], BF16, tag=f"v{st}", name=f"v{st}")
            for st in range(S_TILES)
        ]
        with ExitStack() as p2kv:
            pkv_pool = p2kv.enter_context(tc.tile_pool(name="pkv", bufs=4, space="PSUM"))
            for h in range(HL):
                for skb in range(NSQB):
                    pk = pkv_pool.tile([128, SQB], F32, tag="pkv", name="pk")
                    for kt in range(CKV_TILES):
                        nc.tensor.matmul(
                            pk[:],
                            wkvb_sb[kt][
                                :, h * (NOPE + V_DIM) : h * (NOPE + V_DIM) + NOPE
                            ],
                            latkv_sb[kt][:, skb * SQB : (skb + 1) * SQB],
                            start=(kt == 0),
                            stop=(kt == CKV_TILES - 1),
                        )
                    nc.scalar.copy(knopeT[h][:, skb * SQB : (skb + 1) * SQB], pk[:])
            for st in range(S_TILES):
                pv = pkv_pool.tile([128, HL * V_DIM], F32, tag="pkv", name="pv")
                for kt in range(CKV_TILES):
                    rhs = wkvb_sb[kt][:].rearrange("c (h d) -> c h d", h=HL)[:, :, NOPE:]
                    nc.tensor.matmul(
                        pv[:],
                        latkv_sb[kt][:, st * 128 : (st + 1) * 128],
                        rhs,
                        start=(kt == 0),
                        stop=(kt == CKV_TILES - 1),
                    )
                nc.scalar.copy(v_sb[st][:], pv[:])

        outT_pool = stack.enter_context(tc.tile_pool(name="outT", bufs=1))
        outT = [
            outT_pool.tile([128, S], BF16, tag=f"outT{h}", name=f"outT{h}")
            for h in range(HL)
        ]
        wo_pool = stack.enter_context(tc.tile_pool(name="wo", bufs=1))
        wo_sb = [
            wo_pool.tile([128, HID], BF16, tag=f"wo{h}", name=f"wo{h}")
            for h in range(HL)
        ]
        for h in range(HL):
            nc.scalar.dma_start(wo_sb[h][:], wo_d[h * 128 : (h + 1) * 128, :])

        # ---------------- attention (causal, block-skipped, column-sliced)
        # Per score tile: nope+rope matmuls and the AV matmul stream only the
        # live (causal) column range.  exp tiles are summed in groups of 4 on
        # the DVE; one ones-matmul per group accumulates the softmax
        # denominator.  AV matmuls are deferred 2 tiles so the PE never waits
        # on the DVE-mask -> ACT-exp chain; the renormalization (fp16
        # reciprocal broadcast) is deferred by one (head, sq-block) pair.
        with ExitStack() as p2a:
            am_pool = p2a.enter_context(tc.tile_pool(name="am", bufs=1))
            mask_sb = am_pool.tile([128, 4 * SQB], F32, tag="mask", name="mask")
            for d in range(4):
                nc.gpsimd.dma_start(mask_sb[:, d * SQB : (d + 1) * SQB], mask_d[d])

            ps_pool = p2a.enter_context(tc.tile_pool(name="ps", bufs=4, space="PSUM"))
            psum_sum_pool = p2a.enter_context(
                tc.tile_pool(name="psums", bufs=2, space="PSUM")
            )
            psum_o_pool = p2a.enter_context(
                tc.tile_pool(name="psumo", bufs=2, space="PSUM")
            )
            a_pool = p2a.enter_context(tc.tile_pool(name="apool", bufs=8))
            sg_pool = p2a.enter_context(tc.tile_pool(name="sgpool", bufs=3))
            bc_pool = p2a.enter_context(tc.tile_pool(name="bcpool", bufs=2))

            av_q = []  # score tiles awaiting their AV matmul
            rs_q = []  # groups awaiting the denominator ones-matmul
            ep_q = []  # pairs awaiting the renormalization epilogue

            def drain_av():
                a, h, tk, nk, po, lv = av_q.pop(0)
                nc.tensor.matmul(
                    po[:, lv:],
                    v_sb[tk][:, h * V_DIM : (h + 1) * V_DIM],
                    a[:, lv:],
                    start=(tk == 0),
                    stop=(tk == nk - 1),
                )

            def drain_rs():
                s_g, h, bq, g, psum, po = rs_q.pop(0)
                nc.tensor.matmul(
                    psum[:], ones_sb[:], s_g[:], start=(g == 0), stop=(g == bq)
                )
                if g == bq:
                    # copy the denominators out of PSUM first (fast, frees the
                    # bank) -- the [1,512] reciprocal runs 2.7us on one DVE lane
                    sum_sb = bc_pool.tile([1, SQB], F32, tag="sums", name="sums")
                    nc.scalar.copy(sum_sb[:], psum[:])
                    rs = bc_pool.tile([1, SQB], F16, tag="rs", name="rs")
                    with nc.allow_low_precision(
                        reason="softmax denom reciprocal broadcast in fp16"
                    ):
                        nc.vector.reciprocal(rs[:], sum_sb[:])
                    ep_q.append((h, bq, po, rs))

            def drain_epilogue():
                h, bq, po, rs = ep_q.pop(0)
                bc_ps = ps_pool.tile([128, SQB], F32, tag="ps", name="bc_ps")
                nc.tensor.matmul(bc_ps[:], onesr_sb[:], rs[:], start=True, stop=True)
                bc_sb = bc_pool.tile([128, SQB], F32, tag="bc", name="bc_sb")
                nc.scalar.copy(bc_sb[:], bc_ps[:])
                nc.vector.tensor_mul(
                    outT[h][:, bq * SQB : (bq + 1) * SQB], po[:], bc_sb[:]
                )

            for h in range(HL):
                for bq in range(NSQB):
                    nk = 4 * (bq + 1)
                    psum = psum_sum_pool.tile([1, SQB], F32, tag="psums", name="psum")
                    po = psum_o_pool.tile([128, SQB], F32, tag="psumo", name="po")
                    a_grp = []
                    s_g = None
                    for tk in range(nk):
                        d = tk - 4 * bq
                        lv = 128 * d if d > 0 else 0
                        ps = ps_pool.tile([128, SQB], F32, tag="ps", name="ps")
                        nc.tensor.matmul(
                            ps[:, lv:],
                            knopeT[h][:, tk * 128 : (tk + 1) * 128],
                            qTA[h][:, bq * SQB + lv : (bq + 1) * SQB],
                            start=True,
                            stop=False,
                        )
                        nc.tensor.matmul(
                            ps[:, lv:],
                            kpeT[:, tk * 128 : (tk + 1) * 128],
                            qTB[h][:, bq * SQB + lv : (bq + 1) * SQB],
                            start=False,
                            stop=True,
                        )
                        if d >= 0:
                            nc.vector.tensor_add(
                                ps[:, lv:],
                                ps[:, lv:],
                                mask_sb[:, d * SQB + lv : (d + 1) * SQB],
                            )
                        a = a_pool.tile([128, SQB], BF16, tag="a", name="a")
                        nc.scalar.activation(
                            a[:, lv:], ps[:, lv:], mybir.ActivationFunctionType.Exp
                        )
                        # ---- DVE group sum for the softmax denominator
                        r = tk % 4
                        if r == 0:
                            a_grp = [(a, lv)]
                        else:
                            a_grp.append((a, lv))
                            if r == 1:
                                s_g = sg_pool.tile(
                                    [128, SQB], BF16, tag="sg", name="sg"
                                )
                                a0, lv0 = a_grp[0]
                                nc.vector.tensor_add(
                                    s_g[:, lv:], a0[:, lv:], a[:, lv:]
                                )
                                if lv > lv0:
                                    nc.vector.tensor_copy(
                                        s_g[:, lv0:lv], a0[:, lv0:lv]
                                    )
                            else:
                                nc.vector.tensor_add(
                                    s_g[:, lv:], s_g[:, lv:], a[:, lv:]
                                )
                        av_q.append((a, h, tk, nk, po, lv))
                        if r == 3:
                            rs_q.append((s_g, h, bq, tk // 4, psum, po))
                        while len(av_q) > 2:
                            drain_av()
                        while len(rs_q) > 1:
                            drain_rs()
                        while len(ep_q) > 1:
                            drain_epilogue()
            while av_q:
                drain_av()
            while rs_q:
                drain_rs()
            while ep_q:
                drain_epilogue()

        # ---------------- o_proj (partial: summed across cores on host)
        with ExitStack() as p2o:
            pe_pool = p2o.enter_context(tc.tile_pool(name="pe", bufs=4, space="PSUM"))
            stage_pool = p2o.enter_context(tc.tile_pool(name="stage", bufs=3))
            for st in range(S_TILES):
                for half in range(2):
                    stg = stage_pool.tile([128, 4 * EB], BF16, tag="stage", name="stg")
                    for ebl in range(4):
                        eb = half * 4 + ebl
                        pe = pe_pool.tile([128, EB], F32, tag="pe", name="pe")
                        for h in range(HL):
                            nc.tensor.matmul(
                                pe[:],
                                outT[h][:, st * 128 : (st + 1) * 128],
                                wo_sb[h][:, eb * EB : (eb + 1) * EB],
                                start=(h == 0),
                                stop=(h == HL - 1),
                            )
                        nc.vector.tensor_copy(
                            stg[:, ebl * EB : (ebl + 1) * EB], pe[:]
                        )
                    nc.gpsimd.dma_start(
                        out_d[
                            st * 128 : (st + 1) * 128,
                            half * 4 * EB : (half + 1) * 4 * EB,
                        ],
                        stg[:],
                    )

    nc.compile()
    return nc


_PROGRAM_CACHE = {}


def _get_program() -> bass.Bass:
    if "nc" not in _PROGRAM_CACHE:
        _PROGRAM_CACHE["nc"] = build_program()
    return _PROGRAM_CACHE["nc"]


def _make_mask() -> np.ndarray:
    # mask[d, p, f] for diagonal score tiles: sk-tile tk = 4*bq + d.
    # valid (sq >= sk)  <=>  f >= 128*d + p
    d = np.arange(4)[:, None, None]
    p = np.arange(128)[None, :, None]
    f = np.arange(SQB)[None, None, :]
    return np.where(f >= 128 * d + p, 0.0, NEG).astype(np.float32)


def prepare_inputs(
    hidden_states, w_qkv_a, q_a_gamma, w_q_b, kv_a_gamma, w_kv_b, w_o, b_o
):
    """Host-side prep: fold gammas + attention scale into B weights, cast to
    bf16, repack wa into contiguous tiles, slice per core."""
    bf = ml_dtypes.bfloat16
    hs = np.asarray(hidden_states, np.float32).reshape(S, HID)
    scale = float(Q_HEAD) ** -0.5
    wqb_eff = (
        np.asarray(w_q_b, np.float32)
        * np.asarray(q_a_gamma, np.float32)[:, None]
        * scale
    )
    wkvb_eff = (
        np.asarray(w_kv_b, np.float32) * np.asarray(kv_a_gamma, np.float32)[:, None]
    )
    wa_bf = np.asarray(w_qkv_a, np.float32).astype(bf)
    hs_bf = hs.astype(bf)
    mask = _make_mask()

    # pack wa into one contiguous full-width tile per 128-row band
    wa3 = np.ascontiguousarray(wa_bf.reshape(HT_TILES, 128, C))
    wakv = np.ascontiguousarray(wa3[:, :, Q_LORA:])
    waq = np.ascontiguousarray(wa3[:, :, :Q_LORA])

    wqb_r = wqb_eff.reshape(Q_LORA, H, Q_HEAD)
    wkvb_r = wkvb_eff.reshape(KV_LORA, H, NOPE + V_DIM)
    wo_r = np.asarray(w_o, np.float32).reshape(H, V_DIM, HID)

    in_maps = []
    for c in range(NCORES):
        hsl = np.ascontiguousarray(
            hs_bf[c * SL : (c + 1) * SL]
            .reshape(SL, HT_TILES, 128)
            .transpose(2, 1, 0)
            .reshape(128, HT_TILES * SL)
        )
        wqb_h = wqb_r[:, c * HL : (c + 1) * HL]  # [Q_LORA, 4, 192]
        # column order: h0..h3 nope (128 each), rope pairs (h0h1, h2h3)
        wqb_c = np.ascontiguousarray(
            np.concatenate(
                [
                    wqb_h[:, 0, :NOPE],
                    wqb_h[:, 1, :NOPE],
                    wqb_h[:, 2, :NOPE],
                    wqb_h[:, 3, :NOPE],
                    wqb_h[:, 0, NOPE:],
                    wqb_h[:, 1, NOPE:],
                    wqb_h[:, 2, NOPE:],
                    wqb_h[:, 3, NOPE:],
                ],
                axis=1,
            ).astype(bf)
        )
        wkvb_c = np.ascontiguousarray(
            wkvb_r[:, c * HL : (c + 1) * HL]
            .reshape(KV_LORA, HL * (NOPE + V_DIM))
            .astype(bf)
        )
        wo_c = np.ascontiguousarray(
            wo_r[c * HL : (c + 1) * HL].reshape(HL * V_DIM, HID).astype(bf)
        )
        in_maps.append(
            {
                "hid": hsl,
                "wakv": wakv,
                "waq": waq,
                "wqb": wqb_c,
                "wkvb": wkvb_c,
                "wo": wo_c,
                "mask": mask,
            }
        )
    return in_maps


def kernel(**inputs) -> np.ndarray:
    in_maps = prepare_inputs(**inputs)
    nc = _get_program()
    res = run_bass_kernel_spmd(nc, in_maps, list(range(NCORES)))
    out = np.zeros((S, HID), np.float64)
    for r in res.results:
        out += np.asarray(r["out"], np.float32)
    out = out.astype(np.float32) + np.asarray(inputs["b_o"], np.float32)[None, :]
    return out.reshape(1, S, HID)
